# revision 8
# baseline (speedup 1.0000x reference)
"""Cost-volume kernel for TRN2 (8 NeuronCores, data-parallel over B*H rows).

out[b, 0, d, h, w] = sum_c L[b,c,h,w] * R[b,c,h,(w - d*direction) mod W]

Structure (per core: 96 h-rows, W=640, C=64, D=96):
- Host pre-packs inputs partition-major in fp16 with the 96-col wrap halo
  baked into R, so each row batch needs just two DMAs with ~5 KB
  contiguous segments per partition.
- Rows are processed in pairs: even row's channels in SBUF partitions
  0..63, odd row in 64..127. Matmul issue order alternates the two
  parities so consecutive LDWEIGHTS target the opposite row half of the
  PE array and pull ahead of in-flight matmuls (no row-group conflict).
- Per row, W is split into 20 blocks of 32 columns. Stationary operand =
  L-block [64, 32]; moving operand = R_ext window [64, 128].
  psum[32*ci + i, j] = sum_c L[c, 32a+i] R[c, 32a+j-96], i.e. d = i-j+96
  (j in [i+1, i+96]) -- ~75% of computed dot products are used.
  Col groups ci = tile_position columns; blocks 0..15 of a row pair fill
  one full [128, 512] PSUM bank per parity, blocks 16..19 a second
  [128, 128] tile, so PSUM->SBUF fp16 copies are few and full-partition.
- One contiguous output DMA per row batch on the scalar (ACT) HWDGE ring
  so it never queues behind the input DMAs on the sync ring.
- Host: single as_strided gather undoes the band skew; no per-d rolls.

fp16 in/out (rel err ~6e-4 vs the 2e-2 gate) halves DMA traffic vs fp32.
"""

import os
import numpy as np

import concourse.bacc as bacc
import concourse.bass as bass
import concourse.mybir as mybir
from concourse.bass_utils import run_bass_kernel_spmd
from concourse.tile import TileContext

B, C, H, W = 4, 64, 192, 640
D = 96
EXT = 96                 # left halo: R_ext[x] = R[(x-96) mod W]
NCORES = 8
HS = H // 2              # 96 h-rows per core (shard: b = k//2, h-half = k%2)
WB = 32                  # stationary columns per matmul (w-block)
NB = W // WB             # 20 w-blocks per row
NG = NB // 4             # 5 col-tile groups per row
MV = 128                 # moving columns per matmul
WR = EXT + W             # 736: R_ext width
RB = 8                   # rows per input DMA batch (4 row pairs)
NP = RB // 2             # row pairs per batch
NRB = HS // RB           # 12 row batches

_cache = {}


def _build():
    nc = bacc.Bacc("TRN2", target_bir_lowering=False, debug=False)
    f32 = mybir.dt.float32
    f16 = mybir.dt.float16
    l_sh = nc.dram_tensor("l_sh", [128, NRB, NP, W], f16,
                          kind="ExternalInput")
    r_sh = nc.dram_tensor("r_sh", [128, NRB, NP, WR], f16,
                          kind="ExternalInput")
    # [p, rb, (s par grp j)]: per-partition free block contiguous in DRAM
    g_out = nc.dram_tensor("g_out", [128, NRB, NP * 2 * NG * MV], f16,
                           kind="ExternalOutput")

    with TileContext(nc) as tc:
        with (
            tc.tile_pool(name="inp", bufs=2) as inp,
            tc.tile_pool(name="gst", bufs=2) as gst,
            tc.tile_pool(name="ps", bufs=2, space="PSUM") as ps,
        ):
            for rb in range(NRB):
                lt = inp.tile([128, NP, W], f16, tag="lt")
                rt = inp.tile([128, NP, WR], f16, tag="rt")
                nc.sync.dma_start(out=lt[:], in_=l_sh[:, rb])
                nc.sync.dma_start(out=rt[:], in_=r_sh[:, rb])

                gt = gst.tile([128, NP * 2 * NG * MV], f16, tag="g")
                cpi = 0
                for s in range(NP):
                    # blocks 0..15 -> one full PSUM bank per parity,
                    # blocks 16..19 -> a [128, 128] tile per parity
                    pa = [ps.tile([128, 4 * MV], f32, tag="pae", name="pae"),
                          ps.tile([128, 4 * MV], f32, tag="pao", name="pao")]
                    pb = [ps.tile([128, MV], f32, tag="pbe", name="pbe"),
                          ps.tile([128, MV], f32, tag="pbo", name="pbo")]
                    for a in range(NB):
                        grp, ci = a // 4, a % 4
                        for par in range(2):  # parity-alternating issue
                            pp = slice(64 * par, 64 * par + 64)
                            if grp < 4:
                                dst = pa[par][32 * ci:32 * ci + 32,
                                              grp * MV:(grp + 1) * MV]
                            else:
                                dst = pb[par][32 * ci:32 * ci + 32, :]
                            nc.tensor.matmul(
                                dst,
                                lhsT=lt[pp, s, WB * a:WB * a + WB],
                                rhs=rt[pp, s, WB * a:WB * a + MV],
                                start=True, stop=True,
                                tile_position=(64 * par, 32 * ci))
                    for par in range(2):
                        off = ((s * 2 + par) * NG) * MV
                        for src, width in ((pa[par], 4 * MV), (pb[par], MV)):
                            dst = gt[:, off:off + width]
                            if cpi % 2:
                                nc.vector.tensor_copy(dst, src[:])
                            else:
                                nc.scalar.copy(dst, src[:])
                            cpi += 1
                            off += width
                # output on the ACT HWDGE ring (input uses the sync ring)
                nc.scalar.dma_start(out=g_out[:, rb, :], in_=gt[:])
    nc.finalize()
    return nc


def _get_nc():
    if "nc" not in _cache:
        _cache["nc"] = _build()
    return _cache["nc"]


def _pack(x, width):
    # [64, HS, width] -> [128, NRB, NP, width] fp16, partition-major:
    # out[64*par + c, rb, s] = x[c, rb*RB + 2s + par, :]
    v = np.empty((128, NRB, NP, width), np.float16)
    for par in range(2):
        v[64 * par:64 * par + 64] = x[:, par::2, :].reshape(
            64, NRB, NP, width)
    return v


def kernel(un_l, un_r, direction):
    un_l = np.asarray(un_l)
    un_r = np.asarray(un_r)
    dirv = int(np.asarray(direction))
    assert dirv in (1, -1), f"unsupported direction {dirv}"
    if dirv == -1:
        un_l = un_l[:, :, :, ::-1]
        un_r = un_r[:, :, :, ::-1]
    un_l = np.ascontiguousarray(un_l, dtype=np.float16)
    un_r = np.ascontiguousarray(un_r, dtype=np.float16)

    in_maps = []
    for k in range(NCORES):
        b, hh = k // 2, k % 2
        Lc = un_l[b, :, hh * HS:(hh + 1) * HS, :]
        Rc = un_r[b, :, hh * HS:(hh + 1) * HS, :]
        Rx = np.concatenate([Rc[:, :, W - EXT:], Rc], axis=2)
        in_maps.append({"l_sh": _pack(Lc, W), "r_sh": _pack(Rx, WR)})

    nc = _get_nc()
    trace = bool(int(os.environ.get("CV_TRACE", "0")))
    res = run_bass_kernel_spmd(nc, in_maps, list(range(NCORES)), trace=trace)
    _cache["last_exec_time_ns"] = res.exec_time_ns

    out = np.empty((B, 1, D, H, W), np.float32)
    for k in range(NCORES):
        b, hh = k // 2, k % 2
        gv = res.results[k]["g_out"]  # [128, NRB, NP*2*NG*MV] fp16
        g6 = gv.reshape(4, 32, NRB, NP, 2, NG, MV)  # [ci,i,rb,s,par,grp,j]
        st = g6.strides
        # band[ci, i, rb, s, par, grp, d] = g6[ci, i, rb, s, par, grp, i+96-d]
        band = np.lib.stride_tricks.as_strided(
            g6[:, :, :, :, :, :, EXT:],
            shape=(4, 32, NRB, NP, 2, NG, D),
            strides=(st[0], st[1] + st[6], st[2], st[3], st[4], st[5],
                     -st[6]))
        # out[d, row, w]: row=(rb,s,par), w=(grp,ci,i)
        ovt = band.transpose(6, 2, 3, 4, 5, 0, 1).reshape(D, HS, W)
        dst = out[b, 0, :, hh * HS:(hh + 1) * HS, :]
        dst[...] = ovt
    if dirv == -1:
        out = np.ascontiguousarray(out[:, :, :, :, ::-1])
    return out


# revision 11
# speedup vs baseline: 1.1338x; 1.1338x over previous
"""Cost-volume kernel for TRN2 (8 NeuronCores, data-parallel over B*H rows).

out[b, 0, d, h, w] = sum_c L[b,c,h,w] * R[b,c,h,(w - d*direction) mod W]

Structure (per core: 96 h-rows, W=640, C=64, D=96):
- Host pre-packs inputs partition-major in fp16 with the 96-col wrap halo
  baked into R, so each row batch needs just two DMAs with ~5 KB
  contiguous segments per partition.
- Rows are processed in pairs: even row's channels in SBUF partitions
  0..63, odd row in 64..127. Matmul issue order alternates the two
  parities so consecutive LDWEIGHTS target the opposite row half of the
  PE array and pull ahead of in-flight matmuls (no row-group conflict).
- Per row, W is split into 20 blocks of 32 columns. Stationary operand =
  L-block [64, 32]; moving operand = R_ext window [64, 128].
  psum[32*ci + i, j] = sum_c L[c, 32a+i] R[c, 32a+j-96], i.e. d = i-j+96
  (j in [i+1, i+96]) -- ~75% of computed dot products are used.
  Col groups ci = tile_position columns; blocks 0..15 of a row pair fill
  one full [128, 512] PSUM bank per parity, blocks 16..19 a second
  [128, 128] tile, so PSUM->SBUF fp16 copies are few and full-partition.
- One contiguous output DMA per row batch on the scalar (ACT) HWDGE ring
  so it never queues behind the input DMAs on the sync ring.
- Host: single as_strided gather undoes the band skew; no per-d rolls.

fp16 in/out (rel err ~6e-4 vs the 2e-2 gate) halves DMA traffic vs fp32.
"""

import os
import numpy as np

import concourse.bacc as bacc
import concourse.bass as bass
import concourse.mybir as mybir
from concourse.bass_utils import run_bass_kernel_spmd
from concourse.tile import TileContext

B, C, H, W = 4, 64, 192, 640
D = 96
EXT = 96                 # left halo: R_ext[x] = R[(x-96) mod W]
NCORES = 8
HS = H // 2              # 96 h-rows per core (shard: b = k//2, h-half = k%2)
WB = 32                  # stationary columns per matmul (w-block)
NB = W // WB             # 20 w-blocks per row
NG = NB // 4             # 5 col-tile groups per row
MV = 128                 # moving columns per matmul
WR = EXT + W             # 736: R_ext width
RB = 8                   # rows per input DMA batch (4 row pairs)
NP = RB // 2             # row pairs per batch
NRB = HS // RB           # 12 row batches

_cache = {}


def _build():
    nc = bacc.Bacc("TRN2", target_bir_lowering=False, debug=False)
    f32 = mybir.dt.float32
    f16 = mybir.dt.float16
    l_sh = nc.dram_tensor("l_sh", [128, NRB, NP, W], f16,
                          kind="ExternalInput")
    r_sh = nc.dram_tensor("r_sh", [128, NRB, NP, WR], f16,
                          kind="ExternalInput")
    # [p, rb, (s par grp j)]: per-partition free block contiguous in DRAM
    g_out = nc.dram_tensor("g_out", [128, NRB, NP * 2 * NG * MV], f16,
                           kind="ExternalOutput")

    with TileContext(nc) as tc:
        with (
            tc.tile_pool(name="inp", bufs=3) as inp,
            tc.tile_pool(name="gst", bufs=2) as gst,
            tc.tile_pool(name="ps", bufs=2, space="PSUM") as ps,
        ):
            for rb in range(NRB):
                lt = inp.tile([128, NP, W], f16, tag="lt")
                rt = inp.tile([128, NP, WR], f16, tag="rt")
                nc.sync.dma_start(out=lt[:], in_=l_sh[:, rb])
                nc.sync.dma_start(out=rt[:], in_=r_sh[:, rb])

                gt = gst.tile([128, NP * 2 * NG * MV], f16, tag="g")
                cpi = 0
                for s in range(NP):
                    # blocks 0..15 -> one full PSUM bank per parity,
                    # blocks 16..19 -> a [128, 128] tile per parity
                    pa = [ps.tile([128, 4 * MV], f32, tag="pae", name="pae"),
                          ps.tile([128, 4 * MV], f32, tag="pao", name="pao")]
                    pb = [ps.tile([128, MV], f32, tag="pbe", name="pbe"),
                          ps.tile([128, MV], f32, tag="pbo", name="pbo")]
                    for a in range(NB):
                        grp, ci = a // 4, a % 4
                        for par in range(2):  # parity-alternating issue
                            pp = slice(64 * par, 64 * par + 64)
                            if grp < 4:
                                dst = pa[par][32 * ci:32 * ci + 32,
                                              grp * MV:(grp + 1) * MV]
                            else:
                                dst = pb[par][32 * ci:32 * ci + 32, :]
                            nc.tensor.matmul(
                                dst,
                                lhsT=lt[pp, s, WB * a:WB * a + WB],
                                rhs=rt[pp, s, WB * a:WB * a + MV],
                                start=True, stop=True,
                                tile_position=(64 * par, 32 * ci))
                    for par in range(2):
                        off = ((s * 2 + par) * NG) * MV
                        # balance engines: each gets one big + one small
                        # copy per row pair (gpsimd cannot access PSUM)
                        if cpi % 2:
                            nc.vector.tensor_copy(gt[:, off:off + 4 * MV],
                                                  pa[par][:])
                            nc.scalar.copy(gt[:, off + 4 * MV:off + 5 * MV],
                                           pb[par][:])
                        else:
                            nc.scalar.copy(gt[:, off:off + 4 * MV],
                                           pa[par][:])
                            nc.vector.tensor_copy(
                                gt[:, off + 4 * MV:off + 5 * MV], pb[par][:])
                        cpi += 1
                    # half-batch output DMAs on the ACT HWDGE ring (input
                    # uses the sync ring) so stores overlap compute
                    if s == NP // 2 - 1 or s == NP - 1:
                        hw = NP * NG * MV  # half-batch free width
                        h0 = (0 if s == NP // 2 - 1 else 1) * hw
                        nc.scalar.dma_start(out=g_out[:, rb, h0:h0 + hw],
                                            in_=gt[:, h0:h0 + hw])
    nc.finalize()
    return nc


def _get_nc():
    if "nc" not in _cache:
        _cache["nc"] = _build()
    return _cache["nc"]


def _pack(x, width):
    # [64, HS, width] -> [128, NRB, NP, width] fp16, partition-major:
    # out[64*par + c, rb, s] = x[c, rb*RB + 2s + par, :]
    v = np.empty((128, NRB, NP, width), np.float16)
    for par in range(2):
        v[64 * par:64 * par + 64] = x[:, par::2, :].reshape(
            64, NRB, NP, width)
    return v


def kernel(un_l, un_r, direction):
    un_l = np.asarray(un_l)
    un_r = np.asarray(un_r)
    dirv = int(np.asarray(direction))
    assert dirv in (1, -1), f"unsupported direction {dirv}"
    if dirv == -1:
        un_l = un_l[:, :, :, ::-1]
        un_r = un_r[:, :, :, ::-1]
    un_l = np.ascontiguousarray(un_l, dtype=np.float16)
    un_r = np.ascontiguousarray(un_r, dtype=np.float16)

    in_maps = []
    for k in range(NCORES):
        b, hh = k // 2, k % 2
        Lc = un_l[b, :, hh * HS:(hh + 1) * HS, :]
        Rc = un_r[b, :, hh * HS:(hh + 1) * HS, :]
        Rx = np.concatenate([Rc[:, :, W - EXT:], Rc], axis=2)
        in_maps.append({"l_sh": _pack(Lc, W), "r_sh": _pack(Rx, WR)})

    nc = _get_nc()
    trace = bool(int(os.environ.get("CV_TRACE", "0")))
    res = run_bass_kernel_spmd(nc, in_maps, list(range(NCORES)), trace=trace)
    _cache["last_exec_time_ns"] = res.exec_time_ns

    out = np.empty((B, 1, D, H, W), np.float32)
    for k in range(NCORES):
        b, hh = k // 2, k % 2
        gv = res.results[k]["g_out"]  # [128, NRB, NP*2*NG*MV] fp16
        g6 = gv.reshape(4, 32, NRB, NP, 2, NG, MV)  # [ci,i,rb,s,par,grp,j]
        st = g6.strides
        # band[ci, i, rb, s, par, grp, d] = g6[ci, i, rb, s, par, grp, i+96-d]
        band = np.lib.stride_tricks.as_strided(
            g6[:, :, :, :, :, :, EXT:],
            shape=(4, 32, NRB, NP, 2, NG, D),
            strides=(st[0], st[1] + st[6], st[2], st[3], st[4], st[5],
                     -st[6]))
        # out[d, row, w]: row=(rb,s,par), w=(grp,ci,i)
        ovt = band.transpose(6, 2, 3, 4, 5, 0, 1).reshape(D, HS, W)
        dst = out[b, 0, :, hh * HS:(hh + 1) * HS, :]
        dst[...] = ovt
    if dirv == -1:
        out = np.ascontiguousarray(out[:, :, :, :, ::-1])
    return out
